# revision 1
# baseline (speedup 1.0000x reference)
"""Trainium2 Bass kernel for nn_CustomLSTM: batch-data-parallel LSTM forward.

Contract: kernel(**inputs) takes the FULL unsharded inputs (x [131072,5,30] fp32,
4x W [30,20], 4x U [20,20], 4x b [1,20]) and returns hs [131072,5,20] fp32.

Strategy (pure data parallelism over 8 cores, no cross-device comm):
  - batch B=131072 -> 8 x 16384. Per core, batch index b = q*128 + c where q is
    the SBUF partition and c in [0,128) with c = 4k+g (chunk k, group g).
  - Gate matmuls run batch-major on the PE: stationary = [128,128] bf16 tiles
    holding 4 batch-groups x 32 feature rows; moving = block-diagonal weight
    [128, 320] bf16 (4 copies of the [32,80] augmented weight on the diagonal).
    Biases are folded in via a constant-1.0 feature row.
  - Sigmoid (f,i,o) and tanh (candidate) run on ScalarE straight from PSUM in
    two ops per 4-chunk wave; the cell update runs in bf16 on VectorE (2x mode)
    with gate type-planes for contiguous operands.
  - The recurrent h re-enters the PE feature-major via a DRAM-bounce X-bar
    transpose: the bf16 OUTPUT store doubles as the bounce buffer (slab bytes
    verbatim, re-read as [(q k), 128] blocks), pipelined at half-stage
    granularity so the store+transpose round-trip hides under compute.
  - Outputs land as [T, 2, 2, Q, 8, 128] bf16 and are reassembled + cast to
    fp32 on the host. All x timesteps are SBUF-resident (one early load) and
    the PE/ScalarE are pre-warmed to dodge HAM cold-clock and table loads.
"""

import sys

sys.path.insert(0, "/opt/trn_rl_repo")

import numpy as np
import ml_dtypes

from concourse import bass, bacc, tile, mybir, bass_utils

BF16 = ml_dtypes.bfloat16

N_CORES = 8
B = 131072
T = 5
F = 30
H = 20
BC = B // N_CORES          # 16384 batch per core
Q = 128                    # partitions
C = BC // Q                # 128 slab columns per partition
NK = C // 4                # 32 chunks of 512 batch (4 groups x 128)
FA = 32                    # augmented/padded feature rows per group
NG = 320                   # moving free size: 4 groups x 80 gate columns
NQ = 2                     # pipeline stages per timestep
CQ = C // NQ               # 32 slab columns per quarter
KQ = NK // NQ              # 8 chunks per quarter
WQ = (NK // 4) // NQ       # PSUM waves per stage (4 chunks each)

_nc_cache = {}


def _build_nc():
    if "nc" in _nc_cache:
        return _nc_cache["nc"]
    nc = bacc.Bacc("TRN2", target_bir_lowering=False, debug=False, num_devices=N_CORES)

    xin = nc.dram_tensor("xin", [Q, T, NK, Q], mybir.dt.bfloat16, kind="ExternalInput")
    wblk = nc.dram_tensor("wblk", [Q, NG], mybir.dt.bfloat16, kind="ExternalInput")
    ublk = nc.dram_tensor("ublk", [Q, NG], mybir.dt.bfloat16, kind="ExternalInput")
    # output doubles as the h-transpose bounce: [T, half, q, k, 128] bf16,
    # reassembled + cast to fp32 on the host
    out_d = nc.dram_tensor(
        "out", [T, NQ, 2, Q, KQ // 2, Q], mybir.dt.bfloat16, kind="ExternalOutput"
    )

    SIG = mybir.ActivationFunctionType.Sigmoid
    TANH = mybir.ActivationFunctionType.Tanh
    MUL = mybir.AluOpType.mult
    ADD = mybir.AluOpType.add
    SUB = mybir.AluOpType.subtract

    with tile.TileContext(nc) as tc:
        with (
            tc.tile_pool(name="const", bufs=1) as cpool,
            tc.tile_pool(name="state", bufs=1) as spool,
            tc.tile_pool(name="xload", bufs=1) as xpool,
            tc.tile_pool(name="ht", bufs=2) as hpool,
            tc.tile_pool(name="dram", bufs=1, space="DRAM") as dpool,
            tc.tile_pool(name="gates", bufs=1) as gpool,
            tc.tile_pool(name="cell", bufs=2) as vpool,
            tc.tile_pool(name="psum", bufs=2, space="PSUM") as ppool,
        ):
            wb = cpool.tile([Q, NG], mybir.dt.bfloat16)
            ub = cpool.tile([Q, NG], mybir.dt.bfloat16)
            nc.sync.dma_start(wb[:], wblk[:])
            nc.sync.dma_start(ub[:], ublk[:])

            # prime the ScalarE activation tables (sigmoid/tanh share a set) so
            # the ~2.7us table load runs during the initial x DMA, off the
            # critical path
            actwarm = cpool.tile([Q, 4], mybir.dt.float32)
            nc.gpsimd.memset(actwarm[:], 0.0)
            nc.scalar.activation(
                actwarm[:, 0:2], actwarm[:, 2:4], mybir.ActivationFunctionType.Sigmoid
            )
            nc.scalar.activation(
                actwarm[:, 0:2], actwarm[:, 2:4], mybir.ActivationFunctionType.Tanh
            )

            # persistent state tiles (two slabs alternate per timestep so the
            # bulk output-store read never blocks the next step's h write)
            hslab2 = [
                spool.tile([Q, C, FA], mybir.dt.bfloat16, name=f"hslab{i}")
                for i in range(2)
            ]
            cstate = spool.tile([Q, C, H], mybir.dt.bfloat16)
            # pad columns of hslab multiply zero rows of ublk, but must be finite
            nc.gpsimd.memset(hslab2[0][:], 0.0)
            nc.gpsimd.memset(hslab2[1][:], 0.0)

            # all timesteps of x resident in SBUF (per-t loads so t=0 starts early)
            xall = xpool.tile([Q, T, NK, Q], mybir.dt.bfloat16)
            nc.sync.dma_start(xall[:, 0, 0:8], xin[:, 0, 0:8])
            nc.sync.dma_start(xall[:, 0, 8:NK], xin[:, 0, 8:NK])
            for tl in range(1, T):
                nc.sync.dma_start(xall[:, tl], xin[:, tl])

            # PE warm-up burst: dense matmuls into scratch PSUM while the
            # first x slices load, so t=0 starts at the warm clock
            for wu in range(16):
                wt = ppool.tile([Q, 4, 512], mybir.dt.float32, name="pt")
                for ww in range(4):
                    nc.tensor.matmul(
                        wt[:, ww, 0:NG], wb[:, 0:Q], wb[:], start=True, stop=True
                    )

            hT_next = [None] * (2 * NQ)
            for t in range(T):
                hslab = hslab2[t % 2]
                xbig = xall[:, t]

                for qt in range(NQ):
                    # gate planes for this quarter: f, i, o, ch
                    fio = gpool.tile(
                        [Q, 4, CQ, H], mybir.dt.bfloat16, name=f"fio{qt}",
                        tag=f"fio{qt}", bufs=2,
                    )
                    for wv2 in range(WQ):
                        wv = qt * WQ + wv2
                        pt = ppool.tile([Q, 4, 512], mybir.dt.float32)
                        for w in range(4):
                            k = 4 * wv + w
                            nc.tensor.matmul(
                                pt[:, w, 0:NG],
                                xbig[:, k, :],
                                wb[:],
                                start=True,
                                stop=(t == 0),
                            )
                            if t > 0:
                                kk = k - qt * KQ
                                nc.tensor.matmul(
                                    pt[:, w, 0:NG],
                                    hT_next[2 * qt + kk // (KQ // 2)][
                                        :, :, kk % (KQ // 2)
                                    ],
                                    ub[:],
                                    start=False,
                                    stop=True,
                                )
                        src = pt[:, :, 0:NG].rearrange(
                            "p w (g ty h) -> p w g ty h", g=4, ty=4
                        )
                        dst = fio[:, :, 16 * wv2 : 16 * (wv2 + 1)].rearrange(
                            "p ty (w g) h -> p w g ty h", w=4
                        )
                        ty0 = 1 if t == 0 else 0
                        nc.scalar.activation(
                            dst[:, :, :, ty0:3, :], src[:, :, :, ty0:3, :], SIG
                        )
                        nc.scalar.activation(
                            dst[:, :, :, 3, :], src[:, :, :, 3, :], TANH
                        )

                    # fused cell update, split into two sub-slices so the
                    # store+transpose round-trip pipelines inside the stage
                    CH = CQ // 2
                    for sh in range(2):
                        css = slice(qt * CQ + sh * CH, qt * CQ + (sh + 1) * CH)
                        fs = slice(sh * CH, (sh + 1) * CH)
                        f_ap = fio[:, 0, fs, :]
                        i_ap = fio[:, 1, fs, :]
                        o_ap = fio[:, 2, fs, :]
                        ch_ap = fio[:, 3, fs, :]
                        c_h = cstate[:, css, :]
                        if t == 0:
                            nc.vector.tensor_tensor(c_h, i_ap, ch_ap, MUL)
                        else:
                            t1 = vpool.tile([Q, CH, H], mybir.dt.bfloat16, bufs=4)
                            t2 = vpool.tile([Q, CH, H], mybir.dt.bfloat16, bufs=4)
                            nc.vector.tensor_tensor(t1[:], f_ap, c_h, MUL)
                            nc.vector.tensor_tensor(t2[:], i_ap, ch_ap, MUL)
                            nc.vector.tensor_tensor(c_h, t1[:], t2[:], ADD)
                        tanhc = vpool.tile([Q, CH, H], mybir.dt.bfloat16, bufs=4)
                        nc.scalar.activation(tanhc[:], c_h, TANH)
                        # h -> slab (bf16, strided into the 32-padded rows)
                        nc.vector.tensor_tensor(hslab[:, css, 0:H], o_ap, tanhc[:], MUL)
                        # output store IS the transpose bounce: slab bytes verbatim
                        nc.sync.dma_start(
                            out_d[t][qt][sh][:].rearrange("q k b -> q (k b)"),
                            hslab[:, css, :].rearrange("p a b -> p (a b)"),
                        )
                        if t < T - 1:
                            hT_next[2 * qt + sh] = hpool.tile(
                                [Q, Q, KQ // 2], mybir.dt.bfloat16,
                                name=f"hT{qt}{sh}", tag=f"hT{qt}{sh}",
                            )
                            nc.sync.dma_start(
                                hT_next[2 * qt + sh][:].rearrange("p q k -> p (q k)"),
                                out_d[t][qt][sh][:].rearrange("q k b -> (q k) b"),
                                transpose=True,
                            )

    nc.compile()
    _nc_cache["nc"] = nc
    return nc


def _prep_inputs(x, wf, wi, wo, wc, uf, ui, uo, uc, bf, bi, bo, bc):
    w_all = np.concatenate([wf, wi, wo, wc], axis=1).astype(np.float32)  # [30, 80]
    u_all = np.concatenate([uf, ui, uo, uc], axis=1).astype(np.float32)  # [20, 80]
    b_all = np.concatenate([bf, bi, bo, bc], axis=1).astype(np.float32)  # [1, 80]

    waug = np.zeros((FA, 80), np.float32)
    waug[0:F] = w_all
    waug[F] = b_all[0]
    uaug = np.zeros((FA, 80), np.float32)
    uaug[0:H] = u_all
    wblk = np.zeros((Q, NG), np.float32)
    ublk = np.zeros((Q, NG), np.float32)
    for g in range(4):
        wblk[FA * g : FA * (g + 1), 80 * g : 80 * (g + 1)] = waug
        ublk[FA * g : FA * (g + 1), 80 * g : 80 * (g + 1)] = uaug
    wblk = wblk.astype(BF16)
    ublk = ublk.astype(BF16)

    # x -> per-core stationary layout [T, (g, faug), k, q]
    xr = np.asarray(x, np.float32).reshape(N_CORES, Q, NK, 4, T, F)
    # dims: [core, q, k, g, t, f] -> want [core, t, g, f, k, q]
    xt = xr.transpose(0, 4, 3, 5, 2, 1)
    arr = np.empty((N_CORES, T, 4, FA, NK, Q), np.float32)
    arr[:, :, :, 0:F] = xt
    arr[:, :, :, F] = 1.0
    arr[:, :, :, F + 1 :] = 0.0
    # device wants [Q=(g,faug), T, NK, Q] per core
    arr = arr.transpose(0, 2, 3, 1, 4, 5)  # [core, 4, FA, T, NK, Q]
    xin = np.ascontiguousarray(arr.reshape(N_CORES, Q, T, NK, Q)).astype(BF16)
    return xin, wblk, ublk


LAST_EXEC_NS = None
LAST_RESULTS = None


def kernel(**inputs):
    global LAST_EXEC_NS, LAST_RESULTS
    import os

    nc = _build_nc()
    xin, wblk, ublk = _prep_inputs(**inputs)
    in_maps = [
        {"xin": xin[c], "wblk": wblk, "ublk": ublk} for c in range(N_CORES)
    ]
    kwargs = {}
    if os.environ.get("LSTM_TRACE"):
        kwargs = dict(trace=True, tmpdir=os.environ.get("LSTM_TRACE_DIR") or None)
    try:
        res = bass_utils.run_bass_kernel_spmd(
            nc, in_maps, list(range(N_CORES)), **kwargs
        )
    except Exception:
        # transient device hiccups (e.g. recovering exec units) — retry once
        import time as _time

        _time.sleep(2.0)
        res = bass_utils.run_bass_kernel_spmd(
            nc, in_maps, list(range(N_CORES)), **kwargs
        )
    LAST_EXEC_NS = res.exec_time_ns
    LAST_RESULTS = res
    # device out: [T, half, q, k, (g, f32)] bf16, batch b = q*128 + 64*half + 4k + g
    out = np.empty((B, T, H), np.float32)
    for c in range(N_CORES):
        o = res.results[c]["out"]  # [T, NQ, 2, Q, KQ//2, 128] bf16
        o = o.reshape(T, NQ, 2, Q, KQ // 2, 4, FA)[..., 0:H].astype(np.float32)
        # b = q*128 + qt*64 + sh*32 + 4k + g -> [q, qt, sh, k, g, T, H]
        o = o.transpose(3, 1, 2, 4, 5, 0, 6).reshape(BC, T, H)
        out[c * BC : (c + 1) * BC] = o
    return out

